# revision 1
# baseline (speedup 1.0000x reference)
"""HMM forward-algorithm kernel for Trainium2 (8 NeuronCores, SPMD data-parallel over batch).

Problem: B=64 sequences, T=1024 steps, S=512 states, V=1024 vocab.
  alpha_0 = emission[obs_0] + prior
  alpha_t[b,j] = emission[obs_t][b,j] + logsumexp_i(alpha_{t-1}[b,i] + trans[i,j])
  out[b] = logsumexp_j(alpha_{T-1}[b,j])

Device algorithm (per core, 8 sequences): run the scan in exp-space,
  phi_t[j,b] = (sum_i expT[i,j] * phi_{t-1}[i,b]) * expE_t[j,b] * (periodic rescale)
with phi kept as a [128, 4*8] bf16 SBUF tile (state chunk c, partition p -> state
s = c*128+p; column c*8+b). The 512x512 exp(trans) matrix lives in SBUF as 16
bf16 [128,128] blocks; each step is 16 PE matmuls (K=128, M=128, N=8) accumulated
in PSUM, then one DVE multiply with the pre-gathered emission tile streamed from
DRAM. Every R steps a per-sequence normalizer S1[b] = sum_j phi[j,b] is computed
(GPSIMD partition all-reduce + DVE chunk-reduce), applied LAG steps later as a
multiply by 1/S1, with log(S1) accumulated into a per-sequence log-offset C.
Final answer: C[b] + log(sum_j phi_final[j,b]).

Host side pre-gathers exp(emission_table[obs]) into the packed per-step layout
(pure data movement + exp; the indexing is data-independent of the scan).
"""

import sys

if "/opt/trn_rl_repo" not in sys.path:
    sys.path.insert(0, "/opt/trn_rl_repo")

import numpy as np
import ml_dtypes

import concourse.bass as bass
import concourse.tile as tile
from concourse import bacc
from concourse import mybir

B, T, S, V = 64, 1024, 512, 1024
NCORES = 8
BL = B // NCORES          # 8 sequences per core
NCH = S // 128            # 4 state chunks
PACK = NCH * BL           # 32 packed columns
R_MEAS = 8                # measure normalizer every R steps
LAG = 4                   # apply it this many steps later
DRIFT_COMP = 7.0          # constant log-drift per step, folded into the ES stream
ES_CHUNK = 32             # emission-stream steps per DMA

BF16 = mybir.dt.bfloat16
F32 = mybir.dt.float32


def build_tile_body(tc, w_ap, phi0_ap, es_ap, c0_ap, ones128_ap, sel_ap, ones1_ap, out_ap, n_steps):
    """Emit the full scan. n_steps = number of recurrence steps (T-1)."""
    nc = tc.nc
    import contextlib

    ctx = contextlib.ExitStack()
    with ctx:
        wpool = ctx.enter_context(tc.tile_pool(name="w", bufs=1))
        espool = ctx.enter_context(tc.tile_pool(name="es", bufs=3))
        phipool = ctx.enter_context(tc.tile_pool(name="phi", bufs=3))
        pspool = ctx.enter_context(tc.tile_pool(name="ps", bufs=1, space="PSUM"))
        pssmall = ctx.enter_context(tc.tile_pool(name="pss", bufs=1, space="PSUM"))
        nrmpool = ctx.enter_context(tc.tile_pool(name="nrm", bufs=4))
        accpool = ctx.enter_context(tc.tile_pool(name="acc", bufs=1))

        wt = wpool.tile([128, NCH * NCH * 128], BF16)
        nc.sync.dma_start(wt[:], w_ap[:])

        phi = phipool.tile([128, PACK], BF16, tag="phi")
        nc.sync.dma_start(phi[:], phi0_ap[:])

        cacc = accpool.tile([1, BL], F32)
        nc.sync.dma_start(cacc[:], c0_ap[:])

        ones128_t = accpool.tile([128, 1], BF16, tag="ones128")
        nc.sync.dma_start(ones128_t[:], ones128_ap[:])
        sel_t = accpool.tile([PACK, BL], BF16, tag="sel")
        nc.sync.dma_start(sel_t[:], sel_ap[:])
        ones1_t = accpool.tile([1, 128], BF16, tag="ones1")
        nc.sync.dma_start(ones1_t[:], ones1_ap[:])

        pending = {}  # apply_step -> (rb_tile, lns_tile)
        prev_mm = None

        esc = None
        esc_len = 0
        esc_start = 0

        def col_sums(src_phi, out_dtype):
            """[1, BL] per-sequence sums of src_phi via two PE matmuls."""
            pp = pssmall.tile([PACK, 1], F32, tag="pp")
            nc.tensor.matmul(pp[:], src_phi[:], ones128_t[:], start=True, stop=True)
            ppsb = nrmpool.tile([PACK, 1], BF16, tag="ppsb")
            nc.scalar.copy(ppsb[:], pp[:])
            s1p = pssmall.tile([1, BL], F32, tag="s1p")
            nc.tensor.matmul(s1p[:], ppsb[:], sel_t[:], start=True, stop=True)
            s1b = nrmpool.tile([1, BL], out_dtype, tag="s1b")
            nc.scalar.copy(s1b[:], s1p[:])
            return s1b

        def measure(src_phi, t):
            """rb = bf16(1/S1), lnrb = ln(rb) exactly as applied."""
            s1b = col_sums(src_phi, F32)
            rbf = nrmpool.tile([1, BL], F32, tag="rbf")
            nc.vector.reciprocal(rbf[:], s1b[:])
            rb = nrmpool.tile([1, BL], BF16, tag="rb")
            nc.vector.tensor_copy(rb[:], rbf[:])
            lnrb = nrmpool.tile([1, BL], F32, tag="lnrb")
            nc.scalar.activation(lnrb[:], rb[:], mybir.ActivationFunctionType.Ln)
            return rb, lnrb

        def apply_rescale(dst_phi, rb, lnrb):
            rbB = pssmall.tile([128, PACK], F32, tag="rbB")
            nc.tensor.matmul(
                rbB[:],
                ones1_t[:],
                rb[:, None, :].to_broadcast((1, NCH, BL)),
                start=True,
                stop=True,
            )
            nc.vector.tensor_tensor(
                dst_phi[:], dst_phi[:], rbB[:], mybir.AluOpType.mult
            )
            nc.vector.tensor_sub(cacc[:], cacc[:], lnrb[:])

        for t in range(1, n_steps + 1):
            # emission stream chunk
            idx = t - 1
            if esc is None or idx >= esc_start + esc_len:
                esc_start = idx
                esc_len = min(ES_CHUNK, n_steps - idx)
                esc = espool.tile([128, ES_CHUNK * PACK], BF16, tag="esc")
                nc.sync.dma_start(
                    esc[:, : esc_len * PACK],
                    es_ap[:, esc_start * PACK : (esc_start + esc_len) * PACK],
                )
            off = idx - esc_start

            # one PSUM bank per output chunk, chunk-major matmul order, and a
            # per-chunk DVE multiply: chunk cj's multiply runs while the PE is
            # still working on chunk cj+1, and next step's first matmuls only
            # wait on chunk 0's multiply -- the DVE work hides under PE time.
            # The explicit dep chain pins the scheduler to this PE order
            # (otherwise it round-robins the banks and chunk 0 finishes last).
            newphi = phipool.tile([128, PACK], BF16, tag="phi")
            prev_tt = None
            ps = None
            for cj in range(NCH):
                if cj % 2 == 0:
                    ps = pspool.tile([128, 2 * BL], F32, tag=f"ps{cj // 2}")
                for ci in range(NCH):
                    m = nc.tensor.matmul(
                        ps[:, (cj % 2) * BL : (cj % 2 + 1) * BL],
                        wt[:, (ci * NCH + cj) * 128 : (ci * NCH + cj + 1) * 128],
                        phi[:, ci * BL : (ci + 1) * BL],
                        start=(cj % 2 == 0 and ci == 0),
                        stop=(cj % 2 == 1 and ci == NCH - 1),
                    )
                    if prev_mm is not None:
                        tile.add_dep_helper(m.ins, prev_mm.ins, sync=False, reason="pe order")
                    prev_mm = m
                if cj % 2 == 1:
                    tt = nc.vector.tensor_tensor(
                        newphi[:, (cj - 1) * BL : (cj + 1) * BL],
                        ps[:],
                        esc[:, off * PACK + (cj - 1) * BL : off * PACK + (cj + 1) * BL],
                        mybir.AluOpType.mult,
                    )
                    if prev_tt is not None:
                        tile.add_dep_helper(tt.ins, prev_tt.ins, sync=False, reason="dve order")
                    prev_tt = tt

            # boundary filler: a matmul on the OLD phi keeps the PE queue
            # non-empty while the first chunk multiply of this step finishes,
            # so the next step's first matmul issues back-to-back instead of
            # paying the full ~165ns systolic refill after an idle pipe.
            fill = pssmall.tile([PACK, 1], F32, tag="fill")
            fm = nc.tensor.matmul(fill[:], phi[:], ones128_t[:], start=True, stop=True)
            tile.add_dep_helper(fm.ins, prev_mm.ins, sync=False, reason="pe order")
            prev_mm = fm

            if t in pending:
                rb, lns = pending.pop(t)
                apply_rescale(newphi, rb, lns)

            if t % R_MEAS == 0 and t < n_steps:
                pending[t + LAG] = measure(newphi, t)

            phi = newphi

        # flush remaining rescales into the final phi
        for t in sorted(pending):
            rb, lns = pending.pop(t)
            apply_rescale(phi, rb, lns)

        # final logsumexp: out = C + ln(sum_j phi)
        s1f = col_sums(phi, F32)
        lns = nrmpool.tile([1, BL], F32, tag="lns")
        nc.scalar.activation(lns[:], s1f[:], mybir.ActivationFunctionType.Ln)
        outt = accpool.tile([1, BL], F32, tag="outt")
        nc.vector.tensor_add(outt[:], cacc[:], lns[:])
        nc.sync.dma_start(out_ap[:], outt[:])


def build_program(n_steps, compile=True):
    nc = bacc.Bacc(None)
    w = nc.dram_tensor("w", [128, NCH * NCH * 128], BF16, kind="ExternalInput")
    phi0 = nc.dram_tensor("phi0", [128, PACK], BF16, kind="ExternalInput")
    es = nc.dram_tensor("es", [128, n_steps * PACK], BF16, kind="ExternalInput")
    c0 = nc.dram_tensor("c0", [1, BL], F32, kind="ExternalInput")
    ones128 = nc.dram_tensor("ones128", [128, 1], BF16, kind="ExternalInput")
    sel = nc.dram_tensor("sel", [PACK, BL], BF16, kind="ExternalInput")
    ones1 = nc.dram_tensor("ones1", [1, 128], BF16, kind="ExternalInput")
    out = nc.dram_tensor("out", [1, BL], F32, kind="ExternalOutput")
    with tile.TileContext(nc) as tc:
        build_tile_body(tc, w, phi0, es, c0, ones128, sel, ones1, out, n_steps)
    if compile:
        nc.compile()
    return nc


def host_prepare(observations, emission_table, transitions, prior, n_steps=None):
    """Build per-core input dicts. n_steps defaults to T-1."""
    obs = np.asarray(observations)
    table = np.asarray(emission_table, dtype=np.float32)
    trans = np.asarray(transitions, dtype=np.float32)
    prior = np.asarray(prior, dtype=np.float32)
    Tn = obs.shape[1]
    if n_steps is None:
        n_steps = Tn - 1

    eT = np.exp(trans)
    w = np.empty((128, NCH * NCH * 128), dtype=ml_dtypes.bfloat16)
    for ci in range(NCH):
        for cj in range(NCH):
            w[:, (ci * NCH + cj) * 128 : (ci * NCH + cj + 1) * 128] = eT[
                ci * 128 : (ci + 1) * 128, cj * 128 : (cj + 1) * 128
            ]

    in_maps = []
    for c in range(NCORES):
        bsl = slice(c * BL, (c + 1) * BL)
        E0 = table[obs[bsl, 0]] + prior  # [BL, S]
        c0 = E0.max(axis=1)  # [BL]
        phi0 = np.exp(E0 - c0[:, None])  # [BL, S]
        # pack [BL, S] -> [128, (c b)]
        phi0p = (
            phi0.reshape(BL, NCH, 128).transpose(2, 1, 0).reshape(128, PACK)
        ).astype(ml_dtypes.bfloat16)

        # emission stream for steps 1..n_steps: [128, n_steps*PACK]
        rows = table[obs[bsl, 1 : 1 + n_steps]]  # [BL, n_steps, S]
        ex = np.exp(rows - DRIFT_COMP).reshape(BL, n_steps, NCH, 128)
        esp = (
            ex.transpose(3, 1, 2, 0).reshape(128, n_steps * PACK)
        ).astype(ml_dtypes.bfloat16)

        sel = np.zeros((PACK, BL), dtype=ml_dtypes.bfloat16)
        for cc in range(NCH):
            for b in range(BL):
                sel[cc * BL + b, b] = 1
        in_maps.append(
            {
                "w": w,
                "phi0": phi0p,
                "es": esp,
                "c0": (c0 + DRIFT_COMP * n_steps).reshape(1, BL).astype(np.float32),
                "ones128": np.ones((128, 1), dtype=ml_dtypes.bfloat16),
                "sel": sel,
                "ones1": np.ones((1, 128), dtype=ml_dtypes.bfloat16),
            }
        )
    return in_maps


_CACHE = {}


def _get_program(n_steps):
    if n_steps not in _CACHE:
        _CACHE[n_steps] = build_program(n_steps)
    return _CACHE[n_steps]


def kernel(observations, emission_table, transitions, prior):
    from concourse.bass_utils import run_bass_kernel_spmd

    nc = _get_program(T - 1)
    in_maps = host_prepare(observations, emission_table, transitions, prior)
    res = run_bass_kernel_spmd(nc, in_maps, core_ids=list(range(NCORES)))
    out = np.concatenate([r["out"].reshape(BL) for r in res.results])
    return out.astype(np.float32)



# revision 9
# speedup vs baseline: 1.0130x; 1.0130x over previous
"""HMM forward-algorithm kernel for Trainium2 (8 NeuronCores, SPMD data-parallel over batch).

Problem: B=64 sequences, T=1024 steps, S=512 states, V=1024 vocab.
  alpha_0 = emission[obs_0] + prior
  alpha_t[b,j] = emission[obs_t][b,j] + logsumexp_i(alpha_{t-1}[b,i] + trans[i,j])
  out[b] = logsumexp_j(alpha_{T-1}[b,j])

Device algorithm (per core, 8 sequences): run the scan in exp-space,
  phi_t[j,b] = (sum_i expT[i,j] * phi_{t-1}[i,b]) * expE_t[j,b] * (periodic rescale)
with phi kept as a [128, 4*8] bf16 SBUF tile (state chunk c, partition p -> state
s = c*128+p; column c*8+b). The 512x512 exp(trans) matrix lives in SBUF as 16
bf16 [128,128] blocks; each step is 16 PE matmuls (K=128, M=128, N=8).

The per-step schedule is latency-bound on the cycle
  mult(chunk) -> (PE sem+SBUF refill) -> mms reading that chunk -> psum column
  complete -> (sem+DVE psum access) -> mult(next) ...
so the mm ORDER and the mult GROUPING are chosen (via offline steady-state
simulation) to minimize the exposed round-trip per step: column 3 gets its own
small psum tile + its own early DVE multiply; columns 0-2 share one psum tile
and one multiply at step end. PSUM tiles are double-buffered so step t+1's
matmuls never WAR-stall against step t's multiplies.

Every R steps a per-sequence normalizer S1[b] = sum_j phi[j,b] is computed
(2 PE matmuls), applied LAG steps later as a multiply by 1/S1, with log(S1)
accumulated into a per-sequence log-offset C. Final: C[b] + log(sum_j phi[j,b]).

Host side pre-gathers exp(emission_table[obs]) into the packed per-step layout
(pure data movement + exp; the indexing is data-independent of the scan).
"""

import sys

if "/opt/trn_rl_repo" not in sys.path:
    sys.path.insert(0, "/opt/trn_rl_repo")

import numpy as np
import ml_dtypes

import concourse.bass as bass
import concourse.tile as tile
from concourse import bacc
from concourse import mybir

B, T, S, V = 64, 1024, 512, 1024
NCORES = 8
BL = B // NCORES          # 8 sequences per core
NCH = S // 128            # 4 state chunks
PACK = NCH * BL           # 32 packed columns
R_MEAS = 16               # measure normalizer every R steps
LAG = 4                   # apply it this many steps later
DRIFT_COMP = 7.0          # constant log-drift per step, folded into the ES stream
ES_CHUNK = 32             # emission-stream steps per DMA

BF16 = mybir.dt.bfloat16
F32 = mybir.dt.float32

# (ci, cj) matmul order per step: ci = input chunk (rhs slice of prev phi),
# cj = output column (psum). From offline steady-state search at RT~=405ns.
MM_ORDER = [
    (3, 2), (3, 1), (3, 0), (0, 3),
    (2, 3), (1, 3), (0, 0), (3, 3),
    (1, 0), (2, 1), (0, 2), (1, 1),
    (2, 2), (0, 1), (2, 0), (1, 2),
]
# mult grouping: psB covers output column 3 (multiplied early, it gates the
# (3, *) matmuls of the next step); psA covers columns 0-2.
A_COLS = (0, 1, 2)
B_COLS = (3,)


def build_tile_body(tc, w_ap, phi0_ap, es_ap, c0_ap, ones128_ap, sel_ap, ones1_ap, out_ap, n_steps):
    """Emit the full scan. n_steps = number of recurrence steps (T-1)."""
    nc = tc.nc
    import contextlib

    ctx = contextlib.ExitStack()
    with ctx:
        wpool = ctx.enter_context(tc.tile_pool(name="w", bufs=1))
        espool = ctx.enter_context(tc.tile_pool(name="es", bufs=3))
        phipool = ctx.enter_context(tc.tile_pool(name="phi", bufs=3))
        pspool = ctx.enter_context(tc.tile_pool(name="ps", bufs=2, space="PSUM"))
        pssmall = ctx.enter_context(tc.tile_pool(name="pss", bufs=1, space="PSUM"))
        nrmpool = ctx.enter_context(tc.tile_pool(name="nrm", bufs=4))
        accpool = ctx.enter_context(tc.tile_pool(name="acc", bufs=1))

        wt = wpool.tile([128, NCH * NCH * 128], BF16)
        nc.sync.dma_start(wt[:], w_ap[:])

        phi = phipool.tile([128, PACK], BF16, tag="phi")
        nc.sync.dma_start(phi[:], phi0_ap[:])

        cacc = accpool.tile([1, BL], F32)
        nc.sync.dma_start(cacc[:], c0_ap[:])

        ones128_t = accpool.tile([128, 1], BF16, tag="ones128")
        nc.sync.dma_start(ones128_t[:], ones128_ap[:])
        sel_t = accpool.tile([PACK, BL], BF16, tag="sel")
        nc.sync.dma_start(sel_t[:], sel_ap[:])
        ones1_t = accpool.tile([1, 128], BF16, tag="ones1")
        nc.sync.dma_start(ones1_t[:], ones1_ap[:])

        pending = {}  # apply_step -> (rb_tile, lns_tile)
        prev_mm = None
        prev_tt = None

        esc = None
        esc_len = 0
        esc_start = 0

        nA = len(A_COLS) * BL
        nB = len(B_COLS) * BL
        # column -> (which psum tile, offset within it)
        col_slot = {}
        for k, c in enumerate(A_COLS):
            col_slot[c] = ("A", k * BL)
        for k, c in enumerate(B_COLS):
            col_slot[c] = ("B", k * BL)

        def col_sums(src_phi, out_dtype):
            """[1, BL] per-sequence sums of src_phi via two PE matmuls."""
            nonlocal prev_mm
            pp = pssmall.tile([PACK, 1], F32, tag="pp")
            m = nc.tensor.matmul(pp[:], src_phi[:], ones128_t[:], start=True, stop=True)
            if prev_mm is not None:
                tile.add_dep_helper(m.ins, prev_mm.ins, sync=False, reason="pe order")
            prev_mm = m
            ppsb = nrmpool.tile([PACK, 1], BF16, tag="ppsb")
            nc.scalar.copy(ppsb[:], pp[:])
            s1p = pssmall.tile([1, BL], F32, tag="s1p")
            m = nc.tensor.matmul(s1p[:], ppsb[:], sel_t[:], start=True, stop=True)
            if prev_mm is not None:
                tile.add_dep_helper(m.ins, prev_mm.ins, sync=False, reason="pe order")
            prev_mm = m
            s1b = nrmpool.tile([1, BL], out_dtype, tag="s1b")
            nc.scalar.copy(s1b[:], s1p[:])
            return s1b

        def measure(src_phi, t):
            """rb = bf16(1/S1), lnrb = ln(rb) exactly as applied."""
            s1b = col_sums(src_phi, F32)
            # keep the whole normalizer chain off the DVE FIFO (it would delay
            # the per-step multiplies): rb = exp(-ln(S1)) ~= 1/S1 on ACT only,
            # then lnrb = ln(rb-as-applied) so the compensation stays exact
            lns1 = nrmpool.tile([1, BL], F32, tag="lns1")
            nc.scalar.activation(lns1[:], s1b[:], mybir.ActivationFunctionType.Ln)
            rb = nrmpool.tile([1, BL], BF16, tag="rb")
            nc.scalar.activation(rb[:], lns1[:], mybir.ActivationFunctionType.Exp, scale=-1.0)
            lnrb = nrmpool.tile([1, BL], F32, tag="lnrb")
            nc.scalar.activation(lnrb[:], rb[:], mybir.ActivationFunctionType.Ln)
            return rb, lnrb

        def apply_rescale(dst_phi, rb, lnrb):
            nonlocal prev_mm
            rbB = pssmall.tile([128, PACK], F32, tag="rbB")
            m = nc.tensor.matmul(
                rbB[:],
                ones1_t[:],
                rb[:, None, :].to_broadcast((1, NCH, BL)),
                start=True,
                stop=True,
            )
            if prev_mm is not None:
                tile.add_dep_helper(m.ins, prev_mm.ins, sync=False, reason="pe order")
            prev_mm = m
            nc.vector.tensor_tensor(
                dst_phi[:], dst_phi[:], rbB[:], mybir.AluOpType.mult
            )
            # cacc update on GpSimd: both operands SBUF, keeps DVE FIFO clear
            nc.gpsimd.tensor_sub(cacc[:], cacc[:], lnrb[:])

        for t in range(1, n_steps + 1):
            # emission stream chunk
            idx = t - 1
            if esc is None or idx >= esc_start + esc_len:
                esc_start = idx
                esc_len = min(ES_CHUNK, n_steps - idx)
                esc = espool.tile([128, ES_CHUNK * PACK], BF16, tag="esc")
                nc.sync.dma_start(
                    esc[:, : esc_len * PACK],
                    es_ap[:, esc_start * PACK : (esc_start + esc_len) * PACK],
                )
            off = idx - esc_start

            newphi = phipool.tile([128, PACK], BF16, tag="phi")
            psA = pspool.tile([128, nA], F32, tag="psA", name="psA")
            psB = pspool.tile([128, nB], F32, tag="psB", name="psB") if nB else None

            seenA = 0
            seenB = 0
            for (ci, cj) in MM_ORDER:
                which, coff = col_slot[cj]
                if which == "A":
                    dst = psA[:, coff : coff + BL]
                    start = seenA == 0
                    seenA += 1
                    stop = seenA == 4 * len(A_COLS)
                else:
                    dst = psB[:, coff : coff + BL]
                    start = seenB == 0
                    seenB += 1
                    stop = seenB == 4 * len(B_COLS)
                m = nc.tensor.matmul(
                    dst,
                    wt[:, (ci * NCH + cj) * 128 : (ci * NCH + cj + 1) * 128],
                    phi[:, ci * BL : (ci + 1) * BL],
                    start=start,
                    stop=stop,
                )
                if prev_mm is not None:
                    tile.add_dep_helper(m.ins, prev_mm.ins, sync=False, reason="pe order")
                prev_mm = m

                if which == "B" and seenB == 4 * len(B_COLS):
                    # early multiply for the gating column(s)
                    es_off = off * PACK + B_COLS[0] * BL
                    ttB = nc.vector.tensor_tensor(
                        newphi[:, B_COLS[0] * BL : (B_COLS[0] + len(B_COLS)) * BL],
                        psB[:],
                        esc[:, es_off : es_off + nB],
                        mybir.AluOpType.mult,
                    )
                    if prev_tt is not None:
                        tile.add_dep_helper(ttB.ins, prev_tt.ins, sync=False, reason="dve order")
                    prev_tt = ttB
                if which == "A" and seenA == 4 * len(A_COLS):
                    es_off = off * PACK + A_COLS[0] * BL
                    ttA = nc.vector.tensor_tensor(
                        newphi[:, A_COLS[0] * BL : (A_COLS[0] + len(A_COLS)) * BL],
                        psA[:],
                        esc[:, es_off : es_off + nA],
                        mybir.AluOpType.mult,
                    )
                    if prev_tt is not None:
                        tile.add_dep_helper(ttA.ins, prev_tt.ins, sync=False, reason="dve order")
                    prev_tt = ttA

            if t in pending:
                rb, lns = pending.pop(t)
                apply_rescale(newphi, rb, lns)

            if t % R_MEAS == 0 and t < n_steps:
                pending[t + LAG] = measure(newphi, t)

            phi = newphi

        # flush remaining rescales into the final phi
        for t in sorted(pending):
            rb, lns = pending.pop(t)
            apply_rescale(phi, rb, lns)

        # final logsumexp: out = C + ln(sum_j phi)
        s1f = col_sums(phi, F32)
        lns = nrmpool.tile([1, BL], F32, tag="lns")
        nc.scalar.activation(lns[:], s1f[:], mybir.ActivationFunctionType.Ln)
        outt = accpool.tile([1, BL], F32, tag="outt")
        nc.vector.tensor_add(outt[:], cacc[:], lns[:])
        nc.sync.dma_start(out_ap[:], outt[:])


def build_program(n_steps, compile=True):
    nc = bacc.Bacc(None)
    w = nc.dram_tensor("w", [128, NCH * NCH * 128], BF16, kind="ExternalInput")
    phi0 = nc.dram_tensor("phi0", [128, PACK], BF16, kind="ExternalInput")
    es = nc.dram_tensor("es", [128, n_steps * PACK], BF16, kind="ExternalInput")
    c0 = nc.dram_tensor("c0", [1, BL], F32, kind="ExternalInput")
    ones128 = nc.dram_tensor("ones128", [128, 1], BF16, kind="ExternalInput")
    sel = nc.dram_tensor("sel", [PACK, BL], BF16, kind="ExternalInput")
    ones1 = nc.dram_tensor("ones1", [1, 128], BF16, kind="ExternalInput")
    out = nc.dram_tensor("out", [1, BL], F32, kind="ExternalOutput")
    with tile.TileContext(nc) as tc:
        build_tile_body(tc, w, phi0, es, c0, ones128, sel, ones1, out, n_steps)
    if compile:
        nc.compile()
    return nc


def host_prepare(observations, emission_table, transitions, prior, n_steps=None):
    """Build per-core input dicts. n_steps defaults to T-1."""
    obs = np.asarray(observations)
    table = np.asarray(emission_table, dtype=np.float32)
    trans = np.asarray(transitions, dtype=np.float32)
    prior = np.asarray(prior, dtype=np.float32)
    Tn = obs.shape[1]
    if n_steps is None:
        n_steps = Tn - 1

    eT = np.exp(trans)
    w = np.empty((128, NCH * NCH * 128), dtype=ml_dtypes.bfloat16)
    for ci in range(NCH):
        for cj in range(NCH):
            w[:, (ci * NCH + cj) * 128 : (ci * NCH + cj + 1) * 128] = eT[
                ci * 128 : (ci + 1) * 128, cj * 128 : (cj + 1) * 128
            ]

    in_maps = []
    for c in range(NCORES):
        bsl = slice(c * BL, (c + 1) * BL)
        E0 = table[obs[bsl, 0]] + prior  # [BL, S]
        c0 = E0.max(axis=1)  # [BL]
        phi0 = np.exp(E0 - c0[:, None])  # [BL, S]
        # pack [BL, S] -> [128, (c b)]
        phi0p = (
            phi0.reshape(BL, NCH, 128).transpose(2, 1, 0).reshape(128, PACK)
        ).astype(ml_dtypes.bfloat16)

        # emission stream for steps 1..n_steps: [128, n_steps*PACK]
        rows = table[obs[bsl, 1 : 1 + n_steps]]  # [BL, n_steps, S]
        ex = np.exp(rows - DRIFT_COMP).reshape(BL, n_steps, NCH, 128)
        esp = (
            ex.transpose(3, 1, 2, 0).reshape(128, n_steps * PACK)
        ).astype(ml_dtypes.bfloat16)

        sel = np.zeros((PACK, BL), dtype=ml_dtypes.bfloat16)
        for cc in range(NCH):
            for b in range(BL):
                sel[cc * BL + b, b] = 1
        in_maps.append(
            {
                "w": w,
                "phi0": phi0p,
                "es": esp,
                "c0": (c0 + DRIFT_COMP * n_steps).reshape(1, BL).astype(np.float32),
                "ones128": np.ones((128, 1), dtype=ml_dtypes.bfloat16),
                "sel": sel,
                "ones1": np.ones((1, 128), dtype=ml_dtypes.bfloat16),
            }
        )
    return in_maps


_CACHE = {}


def _get_program(n_steps):
    if n_steps not in _CACHE:
        _CACHE[n_steps] = build_program(n_steps)
    return _CACHE[n_steps]


def kernel(observations, emission_table, transitions, prior):
    from concourse.bass_utils import run_bass_kernel_spmd

    nc = _get_program(T - 1)
    in_maps = host_prepare(observations, emission_table, transitions, prior)
    res = run_bass_kernel_spmd(nc, in_maps, core_ids=list(range(NCORES)))
    out = np.concatenate([r["out"].reshape(BL) for r in res.results])
    return out.astype(np.float32)


# revision 11
# speedup vs baseline: 1.0265x; 1.0133x over previous
"""HMM forward-algorithm kernel for Trainium2 (8 NeuronCores, SPMD data-parallel over batch).

Problem: B=64 sequences, T=1024 steps, S=512 states, V=1024 vocab.
  alpha_0 = emission[obs_0] + prior
  alpha_t[b,j] = emission[obs_t][b,j] + logsumexp_i(alpha_{t-1}[b,i] + trans[i,j])
  out[b] = logsumexp_j(alpha_{T-1}[b,j])

Device algorithm (per core, 8 sequences): run the scan in exp-space,
  phi_t[j,b] = (sum_i expT[i,j] * phi_{t-1}[i,b]) * expE_t[j,b] * (periodic rescale)
with phi kept as a [128, 4*8] bf16 SBUF tile (state chunk c, partition p -> state
s = c*128+p; column c*8+b). The 512x512 exp(trans) matrix lives in SBUF as 16
bf16 [128,128] blocks; each step is 16 PE matmuls (K=128, M=128, N=8).

The per-step schedule is latency-bound on the cycle
  mult(chunk) -> (PE sem+SBUF refill) -> mms reading that chunk -> psum column
  complete -> (sem+DVE psum access) -> mult(next) ...
so the mm ORDER and the mult GROUPING are chosen (via offline steady-state
simulation) to minimize the exposed round-trip per step: column 3 gets its own
small psum tile + its own early DVE multiply; columns 0-2 share one psum tile
and one multiply at step end. PSUM tiles are double-buffered so step t+1's
matmuls never WAR-stall against step t's multiplies.

Every R steps a per-sequence normalizer S1[b] = sum_j phi[j,b] is computed
(2 PE matmuls), applied LAG steps later as a multiply by 1/S1, with log(S1)
accumulated into a per-sequence log-offset C. Final: C[b] + log(sum_j phi[j,b]).

Host side pre-gathers exp(emission_table[obs]) into the packed per-step layout
(pure data movement + exp; the indexing is data-independent of the scan).
"""

import sys

if "/opt/trn_rl_repo" not in sys.path:
    sys.path.insert(0, "/opt/trn_rl_repo")

import numpy as np
import ml_dtypes

import concourse.bass as bass
import concourse.tile as tile
from concourse import bacc
from concourse import mybir

B, T, S, V = 64, 1024, 512, 1024
NCORES = 8
BL = B // NCORES          # 8 sequences per core
NCH = S // 128            # 4 state chunks
PACK = NCH * BL           # 32 packed columns
R_MEAS = 16               # measure normalizer every R steps
LAG = 4                   # apply it this many steps later
DRIFT_COMP = 7.0          # constant log-drift per step, folded into the ES stream
ES_CHUNK = 32             # emission-stream steps per DMA

BF16 = mybir.dt.bfloat16
F32 = mybir.dt.float32

# (ci, cj) matmul order per step: ci = input chunk (rhs slice of prev phi),
# cj = output column (psum). From offline steady-state search at RT~=405ns.
# B-block first: 8 mms reading chunks {2,3} (gated by multB of the previous
# step), then the A-block reading chunks {0,1} (gated by multA). Within each
# block, the mms feeding columns {2,3} come first so multB issues after only
# 4 A-block mms; multA issues at the end. The A-gated block starts right as
# the B-block drains, so the PE queue never empties and refills stay hidden.
MM_ORDER = [
    (2, 2), (3, 2), (2, 3), (3, 3),
    (2, 0), (3, 0), (2, 1), (3, 1),
    (0, 2), (1, 2), (0, 3), (1, 3),
    (0, 0), (1, 0), (0, 1), (1, 1),
]
A_COLS = (0, 1)
B_COLS = (2, 3)


def build_tile_body(tc, w_ap, phi0_ap, es_ap, c0_ap, ones128_ap, sel_ap, ones1_ap, out_ap, n_steps):
    """Emit the full scan. n_steps = number of recurrence steps (T-1)."""
    nc = tc.nc
    import contextlib

    ctx = contextlib.ExitStack()
    with ctx:
        wpool = ctx.enter_context(tc.tile_pool(name="w", bufs=1))
        espool = ctx.enter_context(tc.tile_pool(name="es", bufs=3))
        phipool = ctx.enter_context(tc.tile_pool(name="phi", bufs=3))
        pspool = ctx.enter_context(tc.tile_pool(name="ps", bufs=2, space="PSUM"))
        pssmall = ctx.enter_context(tc.tile_pool(name="pss", bufs=1, space="PSUM"))
        nrmpool = ctx.enter_context(tc.tile_pool(name="nrm", bufs=4))
        accpool = ctx.enter_context(tc.tile_pool(name="acc", bufs=1))

        wt = wpool.tile([128, NCH * NCH * 128], BF16)
        nc.sync.dma_start(wt[:], w_ap[:])

        phi = phipool.tile([128, PACK], BF16, tag="phi")
        nc.sync.dma_start(phi[:], phi0_ap[:])

        cacc = accpool.tile([1, BL], F32)
        nc.sync.dma_start(cacc[:], c0_ap[:])

        ones128_t = accpool.tile([128, 1], BF16, tag="ones128")
        nc.sync.dma_start(ones128_t[:], ones128_ap[:])
        sel_t = accpool.tile([PACK, BL], BF16, tag="sel")
        nc.sync.dma_start(sel_t[:], sel_ap[:])
        ones1_t = accpool.tile([1, 128], BF16, tag="ones1")
        nc.sync.dma_start(ones1_t[:], ones1_ap[:])

        pending = {}  # apply_step -> (rb_tile, lns_tile)
        prev_mm = None
        prev_tt = None

        esc = None
        esc_len = 0
        esc_start = 0

        nA = len(A_COLS) * BL
        nB = len(B_COLS) * BL
        # column -> (which psum tile, offset within it)
        col_slot = {}
        for k, c in enumerate(A_COLS):
            col_slot[c] = ("A", k * BL)
        for k, c in enumerate(B_COLS):
            col_slot[c] = ("B", k * BL)

        def col_sums(src_phi, out_dtype):
            """[1, BL] per-sequence sums of src_phi via two PE matmuls."""
            nonlocal prev_mm
            pp = pssmall.tile([PACK, 1], F32, tag="pp")
            m = nc.tensor.matmul(pp[:], src_phi[:], ones128_t[:], start=True, stop=True)
            if prev_mm is not None:
                tile.add_dep_helper(m.ins, prev_mm.ins, sync=False, reason="pe order")
            prev_mm = m
            ppsb = nrmpool.tile([PACK, 1], BF16, tag="ppsb")
            nc.scalar.copy(ppsb[:], pp[:])
            s1p = pssmall.tile([1, BL], F32, tag="s1p")
            m = nc.tensor.matmul(s1p[:], ppsb[:], sel_t[:], start=True, stop=True)
            if prev_mm is not None:
                tile.add_dep_helper(m.ins, prev_mm.ins, sync=False, reason="pe order")
            prev_mm = m
            s1b = nrmpool.tile([1, BL], out_dtype, tag="s1b")
            nc.scalar.copy(s1b[:], s1p[:])
            return s1b

        def measure(src_phi, t):
            """rb = bf16(1/S1), lnrb = ln(rb) exactly as applied."""
            s1b = col_sums(src_phi, F32)
            # reciprocal+cast on DVE (tiny [1,8] ops, once per R_MEAS steps);
            # ACT only runs Ln/Copy so its function table is never reloaded
            # (adding Exp here costs a 1283ns ACT_TABLE_LOAD per rescale)
            rbf = nrmpool.tile([1, BL], F32, tag="rbf")
            nc.vector.reciprocal(rbf[:], s1b[:])
            rb = nrmpool.tile([1, BL], BF16, tag="rb")
            nc.vector.tensor_copy(rb[:], rbf[:])
            lnrb = nrmpool.tile([1, BL], F32, tag="lnrb")
            nc.scalar.activation(lnrb[:], rb[:], mybir.ActivationFunctionType.Ln)
            return rb, lnrb

        def apply_rescale(dst_phi, rb, lnrb):
            nonlocal prev_mm
            rbB = pssmall.tile([128, PACK], F32, tag="rbB")
            m = nc.tensor.matmul(
                rbB[:],
                ones1_t[:],
                rb[:, None, :].to_broadcast((1, NCH, BL)),
                start=True,
                stop=True,
            )
            if prev_mm is not None:
                tile.add_dep_helper(m.ins, prev_mm.ins, sync=False, reason="pe order")
            prev_mm = m
            nc.vector.tensor_tensor(
                dst_phi[:], dst_phi[:], rbB[:], mybir.AluOpType.mult
            )
            # cacc update on GpSimd: both operands SBUF, keeps DVE FIFO clear
            nc.gpsimd.tensor_sub(cacc[:], cacc[:], lnrb[:])

        for t in range(1, n_steps + 1):
            # emission stream chunk
            idx = t - 1
            if esc is None or idx >= esc_start + esc_len:
                esc_start = idx
                esc_len = min(ES_CHUNK, n_steps - idx)
                esc = espool.tile([128, ES_CHUNK * PACK], BF16, tag="esc")
                nc.sync.dma_start(
                    esc[:, : esc_len * PACK],
                    es_ap[:, esc_start * PACK : (esc_start + esc_len) * PACK],
                )
            off = idx - esc_start

            newphi = phipool.tile([128, PACK], BF16, tag="phi")
            psA = pspool.tile([128, nA], F32, tag="psA", name="psA")
            psB = pspool.tile([128, nB], F32, tag="psB", name="psB") if nB else None

            seenA = 0
            seenB = 0
            for (ci, cj) in MM_ORDER:
                which, coff = col_slot[cj]
                if which == "A":
                    dst = psA[:, coff : coff + BL]
                    start = seenA == 0
                    seenA += 1
                    stop = seenA == 4 * len(A_COLS)
                else:
                    dst = psB[:, coff : coff + BL]
                    start = seenB == 0
                    seenB += 1
                    stop = seenB == 4 * len(B_COLS)
                m = nc.tensor.matmul(
                    dst,
                    wt[:, (ci * NCH + cj) * 128 : (ci * NCH + cj + 1) * 128],
                    phi[:, ci * BL : (ci + 1) * BL],
                    start=start,
                    stop=stop,
                )
                if prev_mm is not None:
                    tile.add_dep_helper(m.ins, prev_mm.ins, sync=False, reason="pe order")
                prev_mm = m

                if which == "B" and seenB == 4 * len(B_COLS):
                    # early multiply for the gating column(s)
                    es_off = off * PACK + B_COLS[0] * BL
                    ttB = nc.vector.tensor_tensor(
                        newphi[:, B_COLS[0] * BL : (B_COLS[0] + len(B_COLS)) * BL],
                        psB[:],
                        esc[:, es_off : es_off + nB],
                        mybir.AluOpType.mult,
                    )
                    if prev_tt is not None:
                        tile.add_dep_helper(ttB.ins, prev_tt.ins, sync=False, reason="dve order")
                    prev_tt = ttB
                if which == "A" and seenA == 4 * len(A_COLS):
                    es_off = off * PACK + A_COLS[0] * BL
                    ttA = nc.vector.tensor_tensor(
                        newphi[:, A_COLS[0] * BL : (A_COLS[0] + len(A_COLS)) * BL],
                        psA[:],
                        esc[:, es_off : es_off + nA],
                        mybir.AluOpType.mult,
                    )
                    if prev_tt is not None:
                        tile.add_dep_helper(ttA.ins, prev_tt.ins, sync=False, reason="dve order")
                    prev_tt = ttA

            if t in pending:
                rb, lns = pending.pop(t)
                apply_rescale(newphi, rb, lns)

            if t % R_MEAS == 0 and t < n_steps:
                pending[t + LAG] = measure(newphi, t)

            phi = newphi

        # flush remaining rescales into the final phi
        for t in sorted(pending):
            rb, lns = pending.pop(t)
            apply_rescale(phi, rb, lns)

        # final logsumexp: out = C + ln(sum_j phi)
        s1f = col_sums(phi, F32)
        lns = nrmpool.tile([1, BL], F32, tag="lns")
        nc.scalar.activation(lns[:], s1f[:], mybir.ActivationFunctionType.Ln)
        outt = accpool.tile([1, BL], F32, tag="outt")
        nc.vector.tensor_add(outt[:], cacc[:], lns[:])
        nc.sync.dma_start(out_ap[:], outt[:])


def build_program(n_steps, compile=True):
    nc = bacc.Bacc(None)
    w = nc.dram_tensor("w", [128, NCH * NCH * 128], BF16, kind="ExternalInput")
    phi0 = nc.dram_tensor("phi0", [128, PACK], BF16, kind="ExternalInput")
    es = nc.dram_tensor("es", [128, n_steps * PACK], BF16, kind="ExternalInput")
    c0 = nc.dram_tensor("c0", [1, BL], F32, kind="ExternalInput")
    ones128 = nc.dram_tensor("ones128", [128, 1], BF16, kind="ExternalInput")
    sel = nc.dram_tensor("sel", [PACK, BL], BF16, kind="ExternalInput")
    ones1 = nc.dram_tensor("ones1", [1, 128], BF16, kind="ExternalInput")
    out = nc.dram_tensor("out", [1, BL], F32, kind="ExternalOutput")
    with tile.TileContext(nc) as tc:
        build_tile_body(tc, w, phi0, es, c0, ones128, sel, ones1, out, n_steps)
    if compile:
        nc.compile()
    return nc


def host_prepare(observations, emission_table, transitions, prior, n_steps=None):
    """Build per-core input dicts. n_steps defaults to T-1."""
    obs = np.asarray(observations)
    table = np.asarray(emission_table, dtype=np.float32)
    trans = np.asarray(transitions, dtype=np.float32)
    prior = np.asarray(prior, dtype=np.float32)
    Tn = obs.shape[1]
    if n_steps is None:
        n_steps = Tn - 1

    eT = np.exp(trans)
    w = np.empty((128, NCH * NCH * 128), dtype=ml_dtypes.bfloat16)
    for ci in range(NCH):
        for cj in range(NCH):
            w[:, (ci * NCH + cj) * 128 : (ci * NCH + cj + 1) * 128] = eT[
                ci * 128 : (ci + 1) * 128, cj * 128 : (cj + 1) * 128
            ]

    in_maps = []
    for c in range(NCORES):
        bsl = slice(c * BL, (c + 1) * BL)
        E0 = table[obs[bsl, 0]] + prior  # [BL, S]
        c0 = E0.max(axis=1)  # [BL]
        phi0 = np.exp(E0 - c0[:, None])  # [BL, S]
        # pack [BL, S] -> [128, (c b)]
        phi0p = (
            phi0.reshape(BL, NCH, 128).transpose(2, 1, 0).reshape(128, PACK)
        ).astype(ml_dtypes.bfloat16)

        # emission stream for steps 1..n_steps: [128, n_steps*PACK]
        rows = table[obs[bsl, 1 : 1 + n_steps]]  # [BL, n_steps, S]
        ex = np.exp(rows - DRIFT_COMP).reshape(BL, n_steps, NCH, 128)
        esp = (
            ex.transpose(3, 1, 2, 0).reshape(128, n_steps * PACK)
        ).astype(ml_dtypes.bfloat16)

        sel = np.zeros((PACK, BL), dtype=ml_dtypes.bfloat16)
        for cc in range(NCH):
            for b in range(BL):
                sel[cc * BL + b, b] = 1
        in_maps.append(
            {
                "w": w,
                "phi0": phi0p,
                "es": esp,
                "c0": (c0 + DRIFT_COMP * n_steps).reshape(1, BL).astype(np.float32),
                "ones128": np.ones((128, 1), dtype=ml_dtypes.bfloat16),
                "sel": sel,
                "ones1": np.ones((1, 128), dtype=ml_dtypes.bfloat16),
            }
        )
    return in_maps


_CACHE = {}


def _get_program(n_steps):
    if n_steps not in _CACHE:
        _CACHE[n_steps] = build_program(n_steps)
    return _CACHE[n_steps]


def kernel(observations, emission_table, transitions, prior):
    from concourse.bass_utils import run_bass_kernel_spmd

    nc = _get_program(T - 1)
    in_maps = host_prepare(observations, emission_table, transitions, prior)
    res = run_bass_kernel_spmd(nc, in_maps, core_ids=list(range(NCORES)))
    out = np.concatenate([r["out"].reshape(BL) for r in res.results])
    return out.astype(np.float32)
